# revision 14
# baseline (speedup 1.0000x reference)
"""Trainium2 Bass kernel for nn_AMXReversibleLayer.

Reference computation (RevNet-style additive coupling):
    x1, x2 = split(x, 2, axis=-1)      # x: [B, S, 2D] f32, each [B, S, D]
    y = concat([x1, x2 + x1 @ W], -1)  # W: [D, D] f32

Strategy: pure data-parallel. x [8, 32768, 256] is sharded along batch —
one batch element (32768 tokens) per NeuronCore, W replicated. No
collectives. The kernel is memory-bound and the per-core DMA fabric
saturates at ~430 GB/s aggregate (all queues share it), so the whole
game is moving fewer bytes and keeping three balanced DMA streams
saturated end-to-end:

  * The first output half is the passthrough y1 = x1 — the device never
    writes it. The host copies it (bit-exact, from the original f32
    input) while unsharding. Device output is y2 only (8 MB bf16).
  * x2 ships as bf16 (8 MB); x1 ships as fp8-e4m3 (4 MB) since it only
    feeds the matmul whose output h has ~0.23x the magnitude of y2 —
    measured end-to-end rel-err ~6e-3 against the 2e-2 gate. The host
    ships fp8(x1/8) and fp8(8*W): the scales cancel in the matmul (so
    PSUM holds h directly, no descale pass) while keeping both
    operands clear of e4m3's subnormal floor. Per-core traffic:
    12.6 MB in + 8.4 MB out = 21 MB, a ~49 us floor at the ~430 GB/s
    cap.
  * x1 ships PRE-TRANSPOSED and tile-ordered (a sharding/layout choice
    made on the host): x1t[d, j*128 + p] = x1[token p*TPP + j, d]. The
    contraction dim d sits on SBUF partitions straight off the DMA, so
    the device runs NO transposes — TensorE does only the 256
    W-matmuls.
  * Three DMA streams on three independent rings so no single ring's
    FIFO serialization caps throughput: x1 (+W first) on the Sync
    HWDGE ring, x2 on the GpSimd SWDGE ring, y2 stores on the ScalarE
    HWDGE ring. SDMA engines round-robin rings at packet granularity,
    so the 4 KB x1 / 8 KB x2 / 8 KB y2 per-partition packets give the
    streams a ~1:2:2 bandwidth split — matching their byte ratio, so
    all three finish together.
  * The y2 = h + x2 adds are the only full-width compute besides the
    matmuls, and the output stream is paced by them, so they are split
    across engines: 2 of 3 bundles go ScalarE copy (PSUM->SBUF bf16)
    then a VectorE add in 2x packed-bf16 mode (PSUM operands force 1x,
    unit-stride bf16 SBUF unlocks 2x); every 3rd bundle adds straight
    from PSUM on VectorE in 1x mode. That balances ScalarE and VectorE
    at ~0.9 us/bundle, faster than input chunks arrive. Adds write a
    COMPACT bf16 y2 tile so output DMA runs are contiguous on both the
    SBUF and HBM side.

Per-core kernel (Tile framework):
  - Token tiling: partition p owns tokens p*TPP + j (TPP = 256).
    Compute tile j = token j of every partition; all three tensors'
    per-partition DMA runs for a group are contiguous. Group loads are
    issued in ldchunk-token slices (finer for group 0) so compute
    starts as soon as the first slice lands; W is issued FIRST on the
    Sync ring so the matmuls are never gated on it.
  - Per bundle of 8 tiles: 8 matmuls lhsT=x1t[:, tile*128:...] x
    rhs=W8 -> PSUM f32 [t, 8*e]; staged or direct add -> y2 tile; y2
    flushed in half-group slices that trail the adds (quarter-group
    for the first and last groups to start the store stream early and
    shorten the final-flush tail).

Quirk handled by _split_matmul_waits: several walrus ISA structs
(Matmult's LDWEIGHTS uop most importantly) encode only ONE sync-wait
command, and Tile sometimes emits 2+ on one instruction ("Too many
sync wait commands" at codegen). The pass hoists extra waits onto
NoOps injected just before the instruction on the same queue.
"""

import ml_dtypes
import numpy as np

import concourse.bass as bass
import concourse.mybir as mybir
from concourse.bass_utils import run_bass_kernel_spmd
from concourse.tile import TileContext

N_CORES = 8
B, S, TWO_D = 8, 32768, 256
D = 128
P = 128

TOKENS = (B * S) // N_CORES          # tokens per core = 32768
TPP = TOKENS // P                    # tokens per partition = 256

BF16 = mybir.dt.bfloat16
FP8 = mybir.dt.float8e4
NP_BF16 = ml_dtypes.bfloat16
NP_FP8 = mybir.dt.np(FP8)
# x1 ships as fp8(x1/8) and W as fp8(8*W): the scales cancel in the
# matmul, so PSUM holds h directly (no descale pass), while 8*W clears
# e4m3's subnormal floor (|W| ~ 0.02) and x1/8 stays in normal range
# for all but sub-0.13-sigma values whose absolute error is negligible.
X1_SCALE = 0.125

_CACHE = {}


def _build_nc(
    tpg: int = 64,                   # tokens per partition per group
    in_bufs: int = 3,
    out_bufs: int = 3,
    bundle: int = 8,                 # tiles per PSUM tile ([128, 1024] f32)
    psum_bufs: int = 4,
    out_splits: int = 2,
    ldchunk: int = 32,               # tokens per input-DMA slice
    ldchunk0: int = 16,              # finer slices for group 0 (faster ramp)
    x1_engine: str = "sync",
    x2_engine: str = "gpsimd",
    out_engine: str = "scalar",
) -> bass.Bass:
    ngroups = TPP // tpg
    nc = bass.Bass()
    x1t = nc.dram_tensor("x1t", [D, TOKENS], FP8, kind="ExternalInput")
    x2 = nc.dram_tensor("x2", [TOKENS, D], BF16, kind="ExternalInput")
    w = nc.dram_tensor("weight", [D, D], FP8, kind="ExternalInput")
    out = nc.dram_tensor("out", [TOKENS, D], BF16, kind="ExternalOutput")

    # Token index of (p, j): p*TPP + j. Partition p owns a CONTIGUOUS
    # run of tokens, so every per-partition DMA run below is contiguous
    # — minimal descriptors at full line rate.
    x1g = x1t.rearrange("d (g c) -> g d c", g=ngroups)          # c = tpg*P cols
    x2g = x2.rearrange("(p g t) d -> g p t d", p=P, g=ngroups)
    og = out.rearrange("(p g t) d -> g p t d", p=P, g=ngroups)

    with TileContext(nc) as tc:
        with (
            tc.tile_pool(name="const", bufs=1) as const_pool,
            tc.tile_pool(name="x1", bufs=in_bufs) as x1_pool,
            tc.tile_pool(name="x2", bufs=in_bufs) as x2_pool,
            tc.tile_pool(name="hs", bufs=psum_bufs) as hs_pool,
            tc.tile_pool(name="y2", bufs=out_bufs) as y2_pool,
            tc.tile_pool(name="psH", bufs=psum_bufs, space="PSUM") as psH_pool,
        ):
            x1_eng = getattr(nc, x1_engine)
            x2_eng = getattr(nc, x2_engine)
            o_eng = getattr(nc, out_engine)

            # W first on the x1 ring: it's tiny and everything gates on it.
            w_sb = const_pool.tile([D, D], FP8)
            x1_eng.dma_start(out=w_sb[:], in_=w[:, :])

            def load_group(g):
                # x1's fp8 runs are half the size of x2's bf16 runs; use
                # group-sized x1 DMAs (bigger packets -> fair RR share)
                # except in group 0 where fine slices shorten the ramp.
                chunk = ldchunk0 if g == 0 else ldchunk
                chunk_x1 = ldchunk0 if g == 0 else tpg
                a = x1_pool.tile([P, tpg * D], FP8, tag="x1")
                b = x2_pool.tile([P, tpg * D], BF16, tag="x2")
                b3 = b[:].rearrange("p (t d) -> p t d", d=D)
                for c in range(max(1, tpg // chunk_x1)):
                    c0, c1 = c * chunk_x1 * D, (c + 1) * chunk_x1 * D
                    x1_eng.dma_start(out=a[:, c0:c1], in_=x1g[g][:, c0:c1])
                for c in range(max(1, tpg // chunk)):
                    x2_eng.dma_start(
                        out=b3[:, c * chunk:(c + 1) * chunk],
                        in_=x2g[g][:, c * chunk:(c + 1) * chunk],
                    )
                return a, b3

            for g in range(ngroups):
                x1s, x2s3 = load_group(g)

                y2t = y2_pool.tile([P, tpg * D], BF16, tag="y2")
                y2t3 = y2t[:].rearrange("p (t d) -> p t d", d=D)
                split = tpg // (
                    out_splits * (2 if g in (0, ngroups - 1) else 1)
                )

                for bdl in range(tpg // bundle):
                    pH = psH_pool.tile([P, bundle * D], mybir.dt.float32)
                    for j in range(bundle):
                        col = (bdl * bundle + j) * D
                        nc.tensor.matmul(
                            pH[:, j * D:(j + 1) * D],
                            lhsT=x1s[:, col:col + D],
                            rhs=w_sb[:],
                            start=True,
                            stop=True,
                        )
                    x2v = x2s3[:, bdl * bundle:(bdl + 1) * bundle]
                    pHv = pH[:].rearrange("p (t d) -> p t d", d=D)
                    y2v = y2t3[:, bdl * bundle:(bdl + 1) * bundle]
                    if bdl % 3 == 2:
                        # Direct 1x-mode add from PSUM on VectorE; no
                        # ScalarE staging. Interleaved 1:2 with the
                        # staged form to balance DVE vs ScalarE load.
                        nc.vector.tensor_add(y2v, pHv, x2v)
                    else:
                        hs = hs_pool.tile([P, bundle * D], BF16, tag="hs")
                        nc.scalar.copy(hs[:], pH[:])
                        hsv = hs[:].rearrange("p (t d) -> p t d", d=D)
                        nc.vector.tensor_add(y2v, hsv, x2v)

                    # Flush each finished slice of the group so the out
                    # DMA trails the adds instead of waiting for the
                    # whole group (shorter pipeline tail).
                    tiles_done = (bdl + 1) * bundle
                    if tiles_done % split == 0:
                        h0 = tiles_done - split
                        o_eng.dma_start(
                            out=og[g][:, h0:tiles_done],
                            in_=y2t3[:, h0:tiles_done],
                        )

    _split_matmul_waits(nc)
    return nc


def _split_matmul_waits(nc: bass.Bass) -> None:
    """Several walrus ISA structs (Matmult's LDWEIGHTS uop, DVE
    TensorCopy, ...) encode only ONE sync-wait command; Tile sometimes
    emits 2+ ("Too many sync wait commands"). Hoist all but one wait
    onto standalone NoOps on the same queue right before the
    instruction — queue order makes this equivalent, and the hoisted
    waits are long-satisfied by then (they are stale WAW ticks)."""
    for blk in nc.cur_f.blocks:
        out = []
        for inst in blk.instructions:
            si = inst.sync_info
            if si is not None and si.on_wait and len(si.on_wait) > 1:
                waits = list(si.on_wait)
                for wait in waits[:-1]:
                    out.append(
                        mybir.InstNoOp(
                            name=nc.get_next_instruction_name(),
                            sync_info=mybir.SyncInfo(on_wait=[wait], on_update=[]),
                            engine=inst.engine,
                            bass_nofuse=True,
                        )
                    )
                inst.sync_info = mybir.SyncInfo(
                    on_wait=[waits[-1]], on_update=list(si.on_update or [])
                )
            out.append(inst)
        blk.instructions = out


def _get_nc() -> bass.Bass:
    if "nc" not in _CACHE:
        _CACHE["nc"] = _build_nc()
    return _CACHE["nc"]


def _in_maps(x: np.ndarray, weight: np.ndarray) -> list[dict[str, np.ndarray]]:
    """Shard along batch; quantize x2 to bf16 and x1/W to fp8-e4m3 (W
    pre-scaled by 64 to clear e4m3's subnormal floor; the kernel folds
    the 1/64 back in). x1 lays out transposed + tile-ordered:
    x1t[d, j*P + p] = x1[token p*TPP + j, d] so the contraction dim
    lands on SBUF partitions straight off the DMA."""
    x = np.ascontiguousarray(np.asarray(x, dtype=np.float32))
    weight = np.ascontiguousarray(np.asarray(weight, dtype=np.float32))
    x4 = x.reshape(N_CORES, P, TPP, TWO_D)
    # [core, p, j, d] -> [core, d, j, p]
    x1t = np.ascontiguousarray(
        (x4[..., :D] * X1_SCALE).astype(NP_FP8).transpose(0, 3, 2, 1)
    ).reshape(N_CORES, D, TOKENS)
    x2 = np.ascontiguousarray(x4[..., D:].astype(NP_BF16)).reshape(
        N_CORES, TOKENS, D
    )
    wb = (weight / X1_SCALE).astype(NP_FP8)
    return [
        {"x1t": x1t[i], "x2": x2[i], "weight": wb} for i in range(N_CORES)
    ]


def _assemble(x: np.ndarray, results: list[dict[str, np.ndarray]]) -> np.ndarray:
    """Unshard: y1 = x1 copied bit-exact from the f32 input; y2 from the
    device's bf16 output, upcast to f32."""
    x = np.asarray(x, dtype=np.float32).reshape(N_CORES, TOKENS, TWO_D)
    out = np.empty((N_CORES, TOKENS, TWO_D), dtype=np.float32)
    out[:, :, :D] = x[:, :, :D]
    for i in range(N_CORES):
        y2 = np.asarray(results[i]["out"]).view(np.uint16)
        out[i, :, D:] = (y2.astype(np.uint32) << np.uint32(16)).view(np.float32)
    return out.reshape(B, S, TWO_D)


def kernel(x: np.ndarray, weight: np.ndarray) -> np.ndarray:
    nc = _get_nc()
    res = run_bass_kernel_spmd(nc, _in_maps(x, weight), core_ids=list(range(N_CORES)))
    return _assemble(x, res.results)


# revision 17
# speedup vs baseline: 1.0496x; 1.0496x over previous
"""Trainium2 Bass kernel for nn_AMXReversibleLayer.

Reference computation (RevNet-style additive coupling):
    x1, x2 = split(x, 2, axis=-1)      # x: [B, S, 2D] f32, each [B, S, D]
    y = concat([x1, x2 + x1 @ W], -1)  # W: [D, D] f32

Strategy: pure data-parallel. x [8, 32768, 256] is sharded along batch —
one batch element (32768 tokens) per NeuronCore, W replicated. No
collectives. The kernel is memory-bound and the per-core DMA fabric
saturates at ~430 GB/s aggregate (all queues share it), so the whole
game is moving fewer bytes and keeping three balanced DMA streams
saturated end-to-end:

  * The first output half is the passthrough y1 = x1 — the device never
    writes it. The host copies it (bit-exact, from the original f32
    input) while unsharding. Device output is y2 only (8 MB bf16).
  * x2 ships as bf16 (8 MB); x1 ships as fp8-e4m3 (4 MB) since it only
    feeds the matmul whose output h has ~0.23x the magnitude of y2 —
    measured end-to-end rel-err ~6e-3 against the 2e-2 gate. The host
    ships fp8(x1/8) and fp8(8*W): the scales cancel in the matmul (so
    PSUM holds h directly, no descale pass) while keeping both
    operands clear of e4m3's subnormal floor. Per-core traffic:
    12.6 MB in + 8.4 MB out = 21 MB, a ~49 us floor at the ~430 GB/s
    cap.
  * x1 ships PRE-TRANSPOSED and tile-ordered (a sharding/layout choice
    made on the host): x1t[d, j*128 + p] = x1[token p*TPP + j, d]. The
    contraction dim d sits on SBUF partitions straight off the DMA, so
    the device runs NO transposes — TensorE does only the 256
    W-matmuls.
  * Three DMA streams on three independent rings so no single ring's
    FIFO serialization caps throughput: x1 (+W first) on the Sync
    HWDGE ring, x2 on the GpSimd SWDGE ring, y2 stores on the ScalarE
    HWDGE ring. SDMA engines round-robin rings at packet granularity,
    so the 4 KB x1 / 8 KB x2 / 8 KB y2 per-partition packets give the
    streams a ~1:2:2 bandwidth split — matching their byte ratio, so
    all three finish together.
  * The y2 = h + x2 adds are the only full-width compute besides the
    matmuls, and the output stream is paced by them, so they are split
    across engines: 2 of 3 bundles go ScalarE copy (PSUM->SBUF bf16)
    then a VectorE add in 2x packed-bf16 mode (PSUM operands force 1x,
    unit-stride bf16 SBUF unlocks 2x); every 3rd bundle adds straight
    from PSUM on VectorE in 1x mode. That balances ScalarE and VectorE
    at ~0.9 us/bundle, faster than input chunks arrive. Adds write a
    COMPACT bf16 y2 tile so output DMA runs are contiguous on both the
    SBUF and HBM side.

Per-core kernel (Tile framework):
  - Token tiling: partition p owns tokens p*TPP + j (TPP = 256).
    Compute tile j = token j of every partition; all three tensors'
    per-partition DMA runs for a group are contiguous. Group loads are
    issued in ldchunk-token slices (finer for group 0) so compute
    starts as soon as the first slice lands; W is issued FIRST on the
    Sync ring so the matmuls are never gated on it.
  - Per bundle of 8 tiles: 8 matmuls lhsT=x1t[:, tile*128:...] x
    rhs=W8 -> PSUM f32 [t, 8*e]; staged or direct add -> y2 tile; y2
    flushed in half-group slices that trail the adds (quarter-group
    for the first and last groups to start the store stream early and
    shorten the final-flush tail).

Quirk handled by _split_matmul_waits: several walrus ISA structs
(Matmult's LDWEIGHTS uop most importantly) encode only ONE sync-wait
command, and Tile sometimes emits 2+ on one instruction ("Too many
sync wait commands" at codegen). The pass hoists extra waits onto
NoOps injected just before the instruction on the same queue.
"""

import ml_dtypes
import numpy as np

import concourse.bass as bass
import concourse.mybir as mybir
from concourse.bass_utils import run_bass_kernel_spmd
from concourse.tile import TileContext

N_CORES = 8
B, S, TWO_D = 8, 32768, 256
D = 128
P = 128

TOKENS = (B * S) // N_CORES          # tokens per core = 32768
TPP = TOKENS // P                    # tokens per partition = 256

BF16 = mybir.dt.bfloat16
FP8 = mybir.dt.float8e4
NP_BF16 = ml_dtypes.bfloat16
NP_FP8 = mybir.dt.np(FP8)
# x1 ships as fp8(x1/8) and W as fp8(8*W): the scales cancel in the
# matmul, so PSUM holds h directly (no descale pass), while 8*W clears
# e4m3's subnormal floor (|W| ~ 0.02) and x1/8 stays in normal range
# for all but sub-0.13-sigma values whose absolute error is negligible.
X1_SCALE = 0.125

_CACHE = {}


def _build_nc(
    tpg: int = 64,                   # tokens per partition per group
    in_bufs: int = 3,
    out_bufs: int = 3,
    bundle: int = 8,                 # tiles per PSUM tile ([128, 1024] f32)
    psum_bufs: int = 4,
    out_splits: int = 2,
    ldchunk: int = 32,               # tokens per input-DMA slice
    ldchunk0: int = 8,               # finer slices for group 0 (faster ramp)
    x1_engine: str = "sync",
    x2_engine: str = "gpsimd",
    out_engine: str = "scalar",
) -> bass.Bass:
    ngroups = TPP // tpg
    nc = bass.Bass()
    x1t = nc.dram_tensor("x1t", [D, TOKENS], FP8, kind="ExternalInput")
    x2 = nc.dram_tensor("x2", [TOKENS, D], BF16, kind="ExternalInput")
    w = nc.dram_tensor("weight", [D, D], FP8, kind="ExternalInput")
    out = nc.dram_tensor("out", [TOKENS, D], BF16, kind="ExternalOutput")

    # Token index of (p, j): p*TPP + j. Partition p owns a CONTIGUOUS
    # run of tokens, so every per-partition DMA run below is contiguous
    # — minimal descriptors at full line rate.
    x1g = x1t.rearrange("d (g c) -> g d c", g=ngroups)          # c = tpg*P cols
    x2g = x2.rearrange("(p g t) d -> g p t d", p=P, g=ngroups)
    og = out.rearrange("(p g t) d -> g p t d", p=P, g=ngroups)

    with TileContext(nc) as tc:
        with (
            tc.tile_pool(name="const", bufs=1) as const_pool,
            tc.tile_pool(name="x1", bufs=in_bufs) as x1_pool,
            tc.tile_pool(name="x2", bufs=in_bufs) as x2_pool,
            tc.tile_pool(name="hs", bufs=psum_bufs) as hs_pool,
            tc.tile_pool(name="y2", bufs=out_bufs) as y2_pool,
            tc.tile_pool(name="psH", bufs=psum_bufs, space="PSUM") as psH_pool,
        ):
            x1_eng = getattr(nc, x1_engine)
            x2_eng = getattr(nc, x2_engine)
            o_eng = getattr(nc, out_engine)

            # W first, on the (otherwise idle-at-start) out ring so it
            # never queues behind x1 data: everything gates on it.
            w_sb = const_pool.tile([D, D], FP8)
            o_eng.dma_start(out=w_sb[:], in_=w[:, :])

            def load_group(g):
                # x1's fp8 runs are half the size of x2's bf16 runs; use
                # group-sized x1 DMAs (bigger packets -> fair RR share)
                # except in group 0 where fine slices shorten the ramp.
                chunk = ldchunk0 if g == 0 else ldchunk
                chunk_x1 = ldchunk0 if g == 0 else tpg
                a = x1_pool.tile([P, tpg * D], FP8, tag="x1")
                b = x2_pool.tile([P, tpg * D], BF16, tag="x2")
                b3 = b[:].rearrange("p (t d) -> p t d", d=D)
                for c in range(max(1, tpg // chunk_x1)):
                    c0, c1 = c * chunk_x1 * D, (c + 1) * chunk_x1 * D
                    x1_eng.dma_start(out=a[:, c0:c1], in_=x1g[g][:, c0:c1])
                for c in range(max(1, tpg // chunk)):
                    x2_eng.dma_start(
                        out=b3[:, c * chunk:(c + 1) * chunk],
                        in_=x2g[g][:, c * chunk:(c + 1) * chunk],
                    )
                return a, b3

            for g in range(ngroups):
                x1s, x2s3 = load_group(g)

                y2t = y2_pool.tile([P, tpg * D], BF16, tag="y2")
                y2t3 = y2t[:].rearrange("p (t d) -> p t d", d=D)
                split = tpg // (
                    out_splits * (2 if g in (0, ngroups - 1) else 1)
                )

                for bdl in range(tpg // bundle):
                    pH = psH_pool.tile([P, bundle * D], mybir.dt.float32)
                    for j in range(bundle):
                        col = (bdl * bundle + j) * D
                        nc.tensor.matmul(
                            pH[:, j * D:(j + 1) * D],
                            lhsT=x1s[:, col:col + D],
                            rhs=w_sb[:],
                            start=True,
                            stop=True,
                        )
                    x2v = x2s3[:, bdl * bundle:(bdl + 1) * bundle]
                    pHv = pH[:].rearrange("p (t d) -> p t d", d=D)
                    y2v = y2t3[:, bdl * bundle:(bdl + 1) * bundle]
                    if bdl % 3 == 2:
                        # Direct 1x-mode add from PSUM on VectorE; no
                        # ScalarE staging. Interleaved 1:2 with the
                        # staged form to balance DVE vs ScalarE load.
                        nc.vector.tensor_add(y2v, pHv, x2v)
                    else:
                        hs = hs_pool.tile([P, bundle * D], BF16, tag="hs")
                        nc.scalar.copy(hs[:], pH[:])
                        hsv = hs[:].rearrange("p (t d) -> p t d", d=D)
                        nc.vector.tensor_add(y2v, hsv, x2v)

                    # Flush each finished slice of the group so the out
                    # DMA trails the adds instead of waiting for the
                    # whole group (shorter pipeline tail). The last two
                    # groups flush on the Sync ring instead: all x1
                    # loads are already queued ahead of them (no
                    # head-of-line risk), and by then x1's stream is
                    # done — two output rings drain the store backlog
                    # in parallel.
                    tiles_done = (bdl + 1) * bundle
                    if tiles_done % split == 0:
                        h0 = tiles_done - split
                        flush_eng = x1_eng if g >= ngroups - 2 else o_eng
                        flush_eng.dma_start(
                            out=og[g][:, h0:tiles_done],
                            in_=y2t3[:, h0:tiles_done],
                        )

    _split_matmul_waits(nc)
    return nc


def _split_matmul_waits(nc: bass.Bass) -> None:
    """Several walrus ISA structs (Matmult's LDWEIGHTS uop, DVE
    TensorCopy, ...) encode only ONE sync-wait command; Tile sometimes
    emits 2+ ("Too many sync wait commands"). Hoist all but one wait
    onto standalone NoOps on the same queue right before the
    instruction — queue order makes this equivalent, and the hoisted
    waits are long-satisfied by then (they are stale WAW ticks)."""
    for blk in nc.cur_f.blocks:
        out = []
        for inst in blk.instructions:
            si = inst.sync_info
            if si is not None and si.on_wait and len(si.on_wait) > 1:
                waits = list(si.on_wait)
                for wait in waits[:-1]:
                    out.append(
                        mybir.InstNoOp(
                            name=nc.get_next_instruction_name(),
                            sync_info=mybir.SyncInfo(on_wait=[wait], on_update=[]),
                            engine=inst.engine,
                            bass_nofuse=True,
                        )
                    )
                inst.sync_info = mybir.SyncInfo(
                    on_wait=[waits[-1]], on_update=list(si.on_update or [])
                )
            out.append(inst)
        blk.instructions = out


def _get_nc() -> bass.Bass:
    if "nc" not in _CACHE:
        _CACHE["nc"] = _build_nc()
    return _CACHE["nc"]


def _in_maps(x: np.ndarray, weight: np.ndarray) -> list[dict[str, np.ndarray]]:
    """Shard along batch; quantize x2 to bf16 and x1/W to fp8-e4m3 (W
    pre-scaled by 64 to clear e4m3's subnormal floor; the kernel folds
    the 1/64 back in). x1 lays out transposed + tile-ordered:
    x1t[d, j*P + p] = x1[token p*TPP + j, d] so the contraction dim
    lands on SBUF partitions straight off the DMA."""
    x = np.ascontiguousarray(np.asarray(x, dtype=np.float32))
    weight = np.ascontiguousarray(np.asarray(weight, dtype=np.float32))
    x4 = x.reshape(N_CORES, P, TPP, TWO_D)
    # [core, p, j, d] -> [core, d, j, p]
    x1t = np.ascontiguousarray(
        (x4[..., :D] * X1_SCALE).astype(NP_FP8).transpose(0, 3, 2, 1)
    ).reshape(N_CORES, D, TOKENS)
    x2 = np.ascontiguousarray(x4[..., D:].astype(NP_BF16)).reshape(
        N_CORES, TOKENS, D
    )
    wb = (weight / X1_SCALE).astype(NP_FP8)
    return [
        {"x1t": x1t[i], "x2": x2[i], "weight": wb} for i in range(N_CORES)
    ]


def _assemble(x: np.ndarray, results: list[dict[str, np.ndarray]]) -> np.ndarray:
    """Unshard: y1 = x1 copied bit-exact from the f32 input; y2 from the
    device's bf16 output, upcast to f32."""
    x = np.asarray(x, dtype=np.float32).reshape(N_CORES, TOKENS, TWO_D)
    out = np.empty((N_CORES, TOKENS, TWO_D), dtype=np.float32)
    out[:, :, :D] = x[:, :, :D]
    for i in range(N_CORES):
        y2 = np.asarray(results[i]["out"]).view(np.uint16)
        out[i, :, D:] = (y2.astype(np.uint32) << np.uint32(16)).view(np.float32)
    return out.reshape(B, S, TWO_D)


def kernel(x: np.ndarray, weight: np.ndarray) -> np.ndarray:
    nc = _get_nc()
    res = run_bass_kernel_spmd(nc, _in_maps(x, weight), core_ids=list(range(N_CORES)))
    return _assemble(x, res.results)
